# revision 1
# baseline (speedup 1.0000x reference)
import sys

sys.path.insert(0, "/opt/trn_rl_repo")

import numpy as np

G, E, N, H = 8, 8192, 512, 32
NP1 = N + 1          # 513
T = N * N            # 262144 tokens per graph
S = 9216             # padded edge slots (72 * 128)
NCHUNK = S // 128    # 72
XG = 8               # token-row groups per input DMA batch
LAG = 6              # stage-2 software-pipeline lag (in groups)


# ----------------------------------------------------------------- device code
def build(nc, outs, ins):
    from contextlib import ExitStack

    import concourse.tile as tile
    from concourse import bass, mybir
    from concourse.masks import make_identity

    f32 = mybir.dt.float32
    fp16 = mybir.dt.float16
    Relu = mybir.ActivationFunctionType.Relu
    Alu = mybir.AluOpType

    out2 = outs["out"]                # [N*H, N] f32, row p*H+h holds (h, p, :)
    emb_out = outs["emb"]             # [S, 32] f32
    xcat = ins["xcat"]                # [114, T] fp16 (rows 0:57 hi, 57:114 lo)
    w1ab = ins["w1ab"]                # [114, 64] fp16 ([A;B])
    w1ba = ins["w1ba"]                # [114, 64] fp16 ([B;A])
    w2ab = ins["w2ab"]                # [128, 32] fp16 ([A2;B2])
    w2ba = ins["w2ba"]                # [128, 32] fp16 ([B2;A2])
    xe6 = ins["xe6"]                  # [6, S] fp16 ([dhi; dlo; dhi; 1; 1; 0])
    w1e6 = ins["w1e6"]                # [6, 32] fp16 ([whi; whi; wlo; bhi; blo; 0])
    w2eab = ins["w2eab"]              # [64, 32] fp16 ([Ae;Be])
    w2eba = ins["w2eba"]              # [64, 32] fp16 ([Be;Ae])
    b2ecol = ins["b2ecol"]            # [32, 1] f32
    table = ins["table"]              # [128, 32]
    tblidx = ins["tblidx"]            # [S] i32
    valids = ins["valids"]            # [S] f32

    with tile.TileContext(nc) as tc, ExitStack() as ctx:
        cst = ctx.enter_context(tc.tile_pool(name="cst", bufs=1))

        # ---- constants
        w1ab_s = cst.tile([114, 64], fp16)
        nc.sync.dma_start(out=w1ab_s[:], in_=w1ab[:])
        w1ba_s = cst.tile([114, 64], fp16)
        nc.sync.dma_start(out=w1ba_s[:], in_=w1ba[:])
        w2ab_s = cst.tile([128, 32], fp16)
        nc.sync.dma_start(out=w2ab_s[:], in_=w2ab[:])
        w2ba_s = cst.tile([128, 32], fp16)
        nc.sync.dma_start(out=w2ba_s[:], in_=w2ba[:])
        w1e_s = cst.tile([6, 32], fp16)
        nc.sync.dma_start(out=w1e_s[:], in_=w1e6[:])
        w2eab_s = cst.tile([64, 32], fp16)
        nc.sync.dma_start(out=w2eab_s[:], in_=w2eab[:])
        w2eba_s = cst.tile([64, 32], fp16)
        nc.sync.dma_start(out=w2eba_s[:], in_=w2eba[:])
        b2e_s = cst.tile([32, 1], f32)
        nc.sync.dma_start(out=b2e_s[:], in_=b2ecol[:])
        ident = cst.tile([32, 32], f32)
        make_identity(nc, ident[:])
        xe_s = cst.tile([6, S], fp16)
        nc.sync.dma_start(out=xe_s[:], in_=xe6[:])
        emb_s = cst.tile([128, NCHUNK * 32], f32)
        nc.sync.dma_start(out=emb_s[:], in_=ins["tblemb"][:])

        # ---- prefetch input batches so their SWDGE dispatches precede the
        # edge pipeline's indirect-DMA dispatches in the GpSimd queue
        xin = ctx.enter_context(tc.tile_pool(name="xin", bufs=8))
        NB = N // XG                  # input batches
        xa_tiles = {}

        def load_batch(b):
            if b >= NB or b in xa_tiles:
                return
            t = xin.tile([114, XG * 512], fp16, tag="xa")
            # SWDGE: descriptors stripe across all 16 DMA engines
            nc.gpsimd.dma_start(out=t[:], in_=xcat[:, b * XG * 512:(b + 1) * XG * 512])
            xa_tiles[b] = t

        for b in range(8):
            load_batch(b)

        # ---- edge embedding pipeline (scoped pools; freed before main loop)
        with ExitStack() as ectx:
            ps_e = ectx.enter_context(tc.tile_pool(name="ps_e", bufs=1, space="PSUM"))
            ps_t = ectx.enter_context(tc.tile_pool(name="ps_t", bufs=2, space="PSUM"))
            ewrk = ectx.enter_context(tc.tile_pool(name="ewrk", bufs=3))
            for c in range(S // 512):
                eh1 = ps_e.tile([32, 512], f32, tag="eh1")
                nc.tensor.matmul(
                    out=eh1[:], lhsT=w1e_s[:], rhs=xe_s[:, c * 512:(c + 1) * 512],
                    start=True, stop=True,
                )
                h1 = ewrk.tile([64, 512], fp16, tag="h1")
                nc.scalar.activation(out=h1[0:32, :], in_=eh1[:], func=Relu)
                nc.vector.scalar_tensor_tensor(
                    out=h1[32:64, :], in0=eh1[:], scalar=0.0,
                    in1=h1[0:32, :], op0=Alu.max, op1=Alu.subtract,
                )
                ed = ps_e.tile([32, 512], f32, tag="ed")
                nc.tensor.matmul(out=ed[:], lhsT=w2eab_s[:], rhs=h1[:],
                                 start=True, stop=False)
                nc.tensor.matmul(out=ed[:], lhsT=w2eba_s[:], rhs=h1[:],
                                 start=False, stop=True)
                demb = ewrk.tile([32, 512], f32, tag="demb")
                nc.vector.tensor_scalar_add(
                    out=demb[:], in0=ed[:], scalar1=b2e_s[:]
                )
                for s4 in range(4):
                    gi = c * 4 + s4
                    etr = ps_t.tile([128, 32], f32, tag="etr")
                    nc.tensor.transpose(
                        out=etr[:], in_=demb[:, s4 * 128:(s4 + 1) * 128],
                        identity=ident[:]
                    )
                    # table rows were preloaded into emb_s host-side;
                    # valid-slot masking is host-side too
                    nc.vector.tensor_add(
                        out=emb_s[:, gi * 32:(gi + 1) * 32], in0=etr[:],
                        in1=emb_s[:, gi * 32:(gi + 1) * 32],
                    )

        # ---- main pass: 512 groups (one output row = 512 tokens each),
        # processed in pairs; stage-2 lagged by LAG groups (LAG/2 pairs)
        hhp = ctx.enter_context(tc.tile_pool(name="hhp", bufs=LAG // 2 + 3))
        osp = ctx.enter_context(tc.tile_pool(name="osp", bufs=3))
        ps1 = ctx.enter_context(tc.tile_pool(name="ps1", bufs=3, space="PSUM"))
        ps2 = ctx.enter_context(tc.tile_pool(name="ps2", bufs=2, space="PSUM"))

        NPAIR = N // 2
        PLAG = LAG // 2
        RL = PLAG + 3
        hh_ring = [None] * RL
        pq4 = None
        osb = None

        for k in range(NPAIR + PLAG):
            if k < NPAIR:
                c0 = 2 * k
                if c0 % XG == 0:
                    load_batch(c0 // XG + 1)
                xa = xa_tiles[c0 // XG]
                off = c0 % XG
                pg2 = ps1.tile([64, 1024], f32, tag="pg")
                for j in (0, 1):
                    rhs = xa[:, (off + j) * 512:(off + j + 1) * 512]
                    nc.tensor.matmul(out=pg2[:, j * 512:(j + 1) * 512],
                                     lhsT=w1ab_s[:], rhs=rhs,
                                     start=True, stop=False)
                    nc.tensor.matmul(out=pg2[:, j * 512:(j + 1) * 512],
                                     lhsT=w1ba_s[:], rhs=rhs,
                                     start=False, stop=True)
                hh2 = hhp.tile([128, 1024], fp16, tag="hh")
                # h_hi = fp16(relu(pg2)); h_lo = fp16(relu(pg2) - h_hi)
                nc.scalar.activation(out=hh2[0:64, :], in_=pg2[:], func=Relu)
                nc.vector.scalar_tensor_tensor(
                    out=hh2[64:128, :], in0=pg2[:], scalar=0.0,
                    in1=hh2[0:64, :], op0=Alu.max, op1=Alu.subtract,
                )
                hh_ring[k % RL] = hh2
            if k >= PLAG:
                kb = k - PLAG
                hhb2 = hh_ring[kb % RL]
                for j in (0, 1):
                    b = 2 * kb + j
                    if b % 4 == 0:
                        pq4 = ps2.tile([128, 512], f32, tag="pq")
                    q0 = (b % 4) * 32
                    rhs2 = hhb2[:, j * 512:(j + 1) * 512]
                    nc.tensor.matmul(out=pq4[q0:q0 + 32, :], lhsT=w2ab_s[:],
                                     rhs=rhs2, start=True, stop=False,
                                     tile_position=(0, q0))
                    nc.tensor.matmul(out=pq4[q0:q0 + 32, :], lhsT=w2ba_s[:],
                                     rhs=rhs2, start=False, stop=True,
                                     tile_position=(0, q0))
                    if b % 4 == 3:
                        p0 = b - 3
                        osb = osp.tile([128, 512], f32, tag="osb")
                        if (b // 4) % 2 == 0:
                            nc.vector.tensor_copy(out=osb[:], in_=pq4[:])
                            dma_eng = nc.scalar
                        else:
                            nc.scalar.copy(out=osb[:], in_=pq4[:])
                            dma_eng = nc.sync
                        dma_eng.dma_start(
                            out=out2[p0 * H:(p0 + 4) * H, :], in_=osb[:]
                        )

        # ---- write edge embeddings back (host applies the scatter-add)
        nc.sync.dma_start(
            out=emb_out.rearrange("(c p) h -> p c h", p=128),
            in_=emb_s[:].rearrange("p (c h) -> p c h", h=32),
        )


# ----------------------------------------------------------------- host prep
def _split16(x):
    hi = x.astype(np.float16)
    lo = (x - hi.astype(np.float32)).astype(np.float16)
    return hi, lo


def prep_core(g, inputs):
    ef = inputs["edge_feat"][g]
    ei = inputs["edge_index"][g].astype(np.int64)
    mask = inputs["edge_mask"][g].astype(bool)
    nlig = max(int(inputs["num_ligand_atoms"][g]), 1)
    angle = inputs["angle"][g]
    dists = inputs["dists"][g]

    x = np.empty((57, T), np.float32)
    x[0:28] = angle.reshape(T, 28).T
    x[28:56] = dists.reshape(T, 28).T
    x[56] = 1.0
    xhi, xlo = _split16(x)
    xcat = np.concatenate([xhi, xlo], 0)            # [114, T] fp16

    w1cat = np.zeros((57, 64), np.float32)
    w1cat[0:28, 0:32] = inputs["ang_w1"]
    w1cat[28:56, 32:64] = inputs["md_w1"]
    w1cat[56, 0:32] = inputs["ang_b1"]
    w1cat[56, 32:64] = inputs["md_b1"]
    a1, b1 = _split16(w1cat)
    w1ab = np.concatenate([a1, b1], 0)              # [114, 64]
    w1ba = np.concatenate([b1, a1], 0)
    w2 = np.concatenate([inputs["ang_w2"], inputs["md_w2"]], 0).astype(np.float32)
    a2, b2 = _split16(w2)
    w2ab = np.concatenate([a2, b2], 0)              # [128, 32]
    w2ba = np.concatenate([b2, a2], 0)
    b2sum = (np.asarray(inputs["ang_b2"]) + np.asarray(inputs["md_b2"])).astype(np.float32)

    t0 = ef[:, 0].astype(np.int64)
    t1 = ef[:, 1].astype(np.int64)
    t2 = ef[:, 2].astype(np.int64)
    d = ef[:, 3].astype(np.float32)
    src, tgt = ei[0], ei[1]
    src_l = (src > 0) & (src < nlig)
    tgt_l = (tgt > 0) & (tgt < nlig)
    structural = t0 <= 1
    plip = t0 == 5
    sidx = np.clip(t0 * 4 + t1 * 2 + t2, 0, 19)
    sel = np.where(src_l & tgt_l, 0, np.where((~src_l) & (~tgt_l), 1, 2))
    pidx = 20 + sel * 15 + np.clip(t1, 0, 14)
    tbl = np.where(structural, sidx, np.where(plip, pidx, 65)).astype(np.int32)
    cell = ((src + 1) * NP1 + (tgt + 1)).astype(np.int64)

    # occurrence rank among valid edges
    rank = np.zeros(E, np.int64)
    vi = np.where(mask)[0]
    cv = cell[vi]
    srt = np.argsort(cv, kind="stable")
    sc = cv[srt]
    first = np.r_[True, sc[1:] != sc[:-1]]
    gstart = np.maximum.accumulate(np.where(first, np.arange(len(sc)), 0))
    rk = np.arange(len(sc)) - gstart
    rv = np.empty(len(cv), np.int64)
    rv[srt] = rk
    rank[vi] = rv

    slot_d = np.zeros(S, np.float32)
    slot_tbl = np.full(S, 65, np.int32)
    slot_cell = np.full(S, -1, np.int64)
    slot_valid = np.zeros(S, bool)

    R0, R1, R2 = 8192, 512, 256
    bounds = [(0, R0), (R0, R0 + R1), (R0 + R1, R0 + R1 + R2), (R0 + R1 + R2, S)]
    cursors = [b[0] for b in bounds]
    for e in range(E):
        r = 0 if not mask[e] else min(int(rank[e]), 3)
        s = cursors[r]
        assert s < bounds[r][1], f"round {r} overflow"
        cursors[r] += 1
        slot_d[s] = d[e]
        slot_tbl[s] = tbl[e]
        slot_cell[s] = cell[e]
        slot_valid[s] = mask[e]

    dhi, dlo = _split16(slot_d)
    xe6 = np.zeros((6, S), np.float16)
    xe6[0] = dhi
    xe6[1] = dlo
    xe6[2] = dhi
    xe6[3] = 1.0
    xe6[4] = 1.0
    we = np.asarray(inputs["dist_w1"], np.float32).reshape(32)
    be = np.asarray(inputs["dist_b1"], np.float32).reshape(32)
    wehi, welo = _split16(we)
    behi, belo = _split16(be)
    w1e6 = np.zeros((6, 32), np.float16)
    w1e6[0] = wehi
    w1e6[1] = wehi
    w1e6[2] = welo
    w1e6[3] = behi
    w1e6[4] = belo
    w2e = np.asarray(inputs["dist_w2"], np.float32)
    ae, be2 = _split16(w2e)
    w2eab = np.concatenate([ae, be2], 0)            # [64, 32]
    w2eba = np.concatenate([be2, ae], 0)
    h_off = (np.arange(H, dtype=np.int64) * (NP1 * NP1))[None, :]
    idx_all = np.where(
        slot_valid[:, None], slot_cell[:, None] + h_off, np.int64(0)
    ).astype(np.int64)

    table = np.zeros((128, 32), np.float32)
    table[0:20] = inputs["struct_emb"]
    table[20:35] = inputs["plip_lig"]
    table[35:50] = inputs["plip_prot"]
    table[50:65] = inputs["plip_inter"]

    tblemb = table[slot_tbl].reshape(NCHUNK, 128, 32).transpose(1, 0, 2)
    tblemb = np.ascontiguousarray(tblemb.reshape(128, NCHUNK * 32), np.float32)

    return dict(
        tblemb=tblemb,
        xcat=xcat,
        w1ab=w1ab,
        w1ba=w1ba,
        w2ab=w2ab,
        w2ba=w2ba,
        xe6=xe6,
        w1e6=w1e6,
        w2eab=w2eab,
        w2eba=w2eba,
        b2ecol=np.asarray(inputs["dist_b2"], np.float32).reshape(32, 1),
        table=table,
        tblidx=slot_tbl,
        valids=slot_valid.astype(np.float32),
    ), b2sum, idx_all


_IN_SPECS = [
    ("xcat", (114, T), "float16"),
    ("w1ab", (114, 64), "float16"),
    ("w1ba", (114, 64), "float16"),
    ("w2ab", (128, 32), "float16"),
    ("w2ba", (128, 32), "float16"),
    ("xe6", (6, S), "float16"),
    ("w1e6", (6, 32), "float16"),
    ("w2eab", (64, 32), "float16"),
    ("w2eba", (64, 32), "float16"),
    ("b2ecol", (32, 1), "float32"),
    ("table", (128, 32), "float32"),
    ("tblemb", (128, NCHUNK * 32), "float32"),
    ("tblidx", (S,), "int32"),
    ("valids", (S,), "float32"),
]


def _build_nc():
    from concourse import bacc, mybir

    nc = bacc.Bacc(
        "TRN2",
        target_bir_lowering=False,
        debug=False,
        enable_asserts=False,
        num_devices=8,
    )
    ins = {}
    for name, shape, dt in _IN_SPECS:
        h = nc.dram_tensor(name, list(shape), getattr(mybir.dt, dt),
                           kind="ExternalInput")
        ins[name] = h[:]
    out_h = nc.dram_tensor("out", [N * H, N], mybir.dt.float32,
                           kind="ExternalOutput")
    emb_h = nc.dram_tensor("emb", [S, 32], mybir.dt.float32, kind="ExternalOutput")
    build(nc, {"out": out_h[:], "emb": emb_h[:]}, ins)
    nc.compile()
    return nc


def kernel(_trace=False, **inputs):
    from concourse.bass_utils import run_bass_kernel_spmd

    in_maps = []
    b2sums = []
    idxs = []
    for g in range(G):
        m, b2sum, idx_all = prep_core(g, inputs)
        in_maps.append(m)
        b2sums.append(b2sum)
        idxs.append(idx_all)

    nc = _build_nc()
    res = run_bass_kernel_spmd(nc, in_maps, core_ids=list(range(G)), trace=_trace)
    if _trace:
        print("HW exec time:", res.exec_time_ns, "ns  (mean:", res.mean_exec_time_ns,
              "ns, slowest core:", res.max_exec_time_core_id, ")")
        if res.instructions_and_trace:
            print("trace:", res.instructions_and_trace[1])

    attn = np.asarray(inputs["attn_bias"], np.float32)      # [G, 513, 513]
    virt = np.asarray(inputs["virt"], np.float32).reshape(H)
    outs = []
    for g, r in enumerate(res.results):
        dev = r["out"].reshape(N, H, N).transpose(1, 0, 2)  # -> [H, N, N]
        full = np.empty((H, NP1, NP1), np.float32)
        full[:, 1:, 1:] = dev + attn[g][None, 1:, 1:]
        full[:, 1:, 0] = attn[g][None, 1:, 0] + virt[:, None]
        full[:, 0, :] = attn[g][None, 0, :] + virt[:, None]
        flat = full.reshape(-1)
        emb = r["emb"] * in_maps[g]["valids"][:, None]
        np.add.at(flat, idxs[g].ravel(), emb.ravel())
        outs.append(flat.reshape(H, NP1, NP1))
    out = np.stack(outs)
    b2s = np.stack(b2sums)  # [G, 32]
    if np.any(b2s != 0):
        out[:, :, 1:, 1:] += b2s[:, :, None, None]
    return out.astype(np.float32)

